# revision 44
# baseline (speedup 1.0000x reference)
"""DepthAttentionResidual Trainium2 kernel (t-on-partitions + diag-matmul mix).

Computation (see reference):
    ms      = mean(history^2, axis=-1)                      # [S,B,T]
    logits  = dot(query*rms_weight, history) * rsqrt(ms+eps)
    w       = softmax(logits, axis=S)
    out     = sum_s w[s] * history[s]                        # [B,T,D]

Sharding: data-parallel over (B=4) x (T halves) = 8 cores. Each core gets
hist [S=16, Tc=1024, D=1024] (64 MiB) and produces out [1024, 1024].

Per-core layout: a supertile is 128 consecutive t positions mapped to the
128 SBUF partitions; the free axis holds (s, d).

DMA: even s ride the SP HWDGE ring as 8 single-slice DMAs whose 128 4-KiB
descriptors each cover one contiguous 512 KiB DRAM span; odd s ride the
ScalarE ring as 2 four-slice DMAs (fewer configs keep the Act engine
free). A single ring caps at ~240 GB/s descriptor dispatch; two rings
measured ~280-400 GB/s combined. Output + constants ride the GpSimd SWDGE
queue as a third stream. DMAs for supertile k+1 are emitted before
compute of supertile k so ring configs never queue behind compute.

Compute (per supertile, consuming even s first, then odd):
  - sum(h^2) over d: ScalarE activation(Square, accum_out), one slice on
    VectorE to balance (~1.2 us per [128,1024] fp32 pass on either)
  - dot(q*w, h) over d: VectorE affine_mul_reduce
  - softmax over s is a free-axis reduction: [128,16] elementwise ops
  - depth mix on PE: psum[t,d] += diag(w_s) @ h_s, 16 accumulating fp32r
    matmuls per 512-wide chunk; diag(w_s) built on GpSimd from an identity
    constant; PSUM -> SBUF on VectorE; one 512 KiB output DMA per supertile
"""
import numpy as np

import concourse.bass as bass
import concourse.bacc as bacc
import concourse.tile as tile
from concourse import mybir
from concourse import bass_utils

N_CORES = 8
S = 16
B = 4
T = 2048
D = 1024
EPS = 1e-5

TC = T // 2          # t positions per core
TS = 128             # t per supertile (= SBUF partitions)
N_SUPER = TC // TS   # supertiles per core = 8
F32 = mybir.dt.float32
F32R = mybir.dt.float32r

# stats consumption order: ring-A slices (even s) first, then ring-B
ORDER = list(range(0, S, 2)) + list(range(1, S, 2))
DVE_SUMSQ = {0}      # sumsq slices on VectorE instead of ScalarE


def _build_program():
    nc = bacc.Bacc("TRN2", target_bir_lowering=False, debug=False,
                   enable_asserts=True, num_devices=N_CORES)

    hist = nc.dram_tensor("hist", [S, TC, D], F32R, kind="ExternalInput").ap()
    query = nc.dram_tensor("query", [D], F32, kind="ExternalInput").ap()
    rmsw = nc.dram_tensor("rms_weight", [D], F32, kind="ExternalInput").ap()
    id_d = nc.dram_tensor("ident", [128, 128], F32, kind="ExternalInput").ap()
    out = nc.dram_tensor("out", [TC, D], F32, kind="ExternalOutput").ap()

    with tile.TileContext(nc) as tc:
        with (
            tc.tile_pool(name="singles", bufs=1) as singles,
            tc.tile_pool(name="hsup", bufs=2) as hpool,
            tc.tile_pool(name="stats", bufs=2) as stats,
            tc.tile_pool(name="diagp", bufs=3) as diagp,
            tc.tile_pool(name="outp", bufs=2) as outpool,
            tc.tile_pool(name="ps_mix", bufs=2, space="PSUM") as ps_mix,
        ):
            qw = singles.tile([128, D], F32)
            wb = singles.tile([128, D], F32)
            ident = singles.tile([128, 128], F32)
            epst = singles.tile([128, 1], F32)
            dummy_a = singles.tile([128, 1], F32)
            dummy_v = singles.tile([128, 1], F32)

            def emit_init():
                # constants ride the SWDGE queue; tiny, done early
                nc.gpsimd.dma_start(
                    out=qw[:],
                    in_=bass.AP(tensor=query.tensor, offset=0,
                                ap=[[0, 128], [1, D]]),
                )
                nc.gpsimd.dma_start(
                    out=wb[:],
                    in_=bass.AP(tensor=rmsw.tensor, offset=0,
                                ap=[[0, 128], [1, D]]),
                )
                nc.gpsimd.dma_start(out=ident[:], in_=id_d)
                nc.vector.tensor_mul(qw[:], qw[:], wb[:])  # query * rms_weight
                nc.vector.memset(epst[:], EPS)

            emit_init()

            hsups = [None] * N_SUPER

            def emit_dma(k):
                t0 = k * TS
                hsupA = hpool.tile([128, S // 2, D], F32R, tag="hsupA",
                                   name=f"hsupA{k}")
                hsupB = hpool.tile([128, S // 2, D], F32R, tag="hsupB",
                                   name=f"hsupB{k}")
                # ring A (SP): even s, one slice per DMA, descriptors walk a
                # contiguous 512 KiB DRAM span
                for s in range(0, S, 2):
                    src = hist[s:s + 1, t0:t0 + TS, :].rearrange(
                        "o t d -> (o t) d")
                    nc.sync.dma_start(out=hsupA[:, s // 2, :], in_=src)
                # ring B (ScalarE): odd s, 4 slices per DMA (2 configs)
                for half in range(2):
                    s0 = 1 + half * 8          # s = s0, s0+2, s0+4, s0+6
                    src = bass.AP(
                        tensor=hist.tensor,
                        offset=s0 * TC * D + t0 * D,
                        ap=[[D, TS], [2 * TC * D, 4], [1, D]],
                    )
                    nc.scalar.dma_start(
                        out=hsupB[:, half * 4:(half + 1) * 4, :], in_=src)
                hsups[k] = (hsupA, hsupB)

            def hslice(k, s):
                hsupA, hsupB = hsups[k]
                return (hsupA if s % 2 == 0 else hsupB)[:, s // 2, :]

            def emit_compute(k):
                t0 = k * TS
                ss = stats.tile([128, S], F32, tag="ss")
                dot = stats.tile([128, S], F32, tag="dot")
                for s in ORDER:
                    h_s = hslice(k, s).bitcast(F32)
                    if s in DVE_SUMSQ:
                        nc.vector.affine_mul_reduce(
                            out=dummy_v.broadcast_to([128, D]),
                            accum_out=ss[:, s:s + 1],
                            in0=h_s, in1=h_s, scale=1.0, bias=0.0,
                        )
                    else:
                        nc.scalar.activation(
                            out=dummy_a.broadcast_to([128, D]),
                            in_=h_s,
                            func=mybir.ActivationFunctionType.Square,
                            accum_out=ss[:, s:s + 1],
                        )
                    nc.vector.affine_mul_reduce(
                        out=dummy_v.broadcast_to([128, D]),
                        accum_out=dot[:, s:s + 1],
                        in0=h_s,
                        in1=qw[:],
                        scale=1.0,
                        bias=0.0,
                    )

                # rstd = 1/sqrt(ss/D + eps); logit = dot * rstd; e = exp
                sd = stats.tile([128, S], F32, tag="sd")
                nc.scalar.activation(
                    out=sd[:], in_=ss[:],
                    func=mybir.ActivationFunctionType.Sqrt,
                    bias=epst[:], scale=1.0 / D,
                )
                rstd = stats.tile([128, S], F32, tag="rstd")
                nc.vector.reciprocal(out=rstd[:], in_=sd[:])
                logit = stats.tile([128, S], F32, tag="logit")
                nc.vector.tensor_mul(logit[:], dot[:], rstd[:])
                e = stats.tile([128, S], F32, tag="e")
                nc.scalar.activation(
                    out=e[:], in_=logit[:],
                    func=mybir.ActivationFunctionType.Exp,
                )
                # sumexp over s (free axis) on VectorE: sum(e * 1)
                se = stats.tile([128, 1], F32, tag="se")
                nc.vector.affine_mul_reduce(
                    out=dummy_v.broadcast_to([128, S]),
                    accum_out=se[:],
                    in0=e[:], in1=e[:], scale=0.0, bias=1.0,
                )
                rse = stats.tile([128, 1], F32, tag="rse")
                nc.vector.reciprocal(out=rse[:], in_=se[:])
                w = stats.tile([128, S], F32, tag="w")
                nc.vector.tensor_scalar(
                    out=w[:], in0=e[:], scalar1=rse[:], scalar2=None,
                    op0=mybir.AluOpType.mult,
                )

                # depth mix on PE: psum[t, d] += diag(w_s) @ h_s
                m_ps = [ps_mix.tile([128, 512], F32, tag="m", name=f"m{c}")
                        for c in range(2)]
                for i, s in enumerate(ORDER):
                    diag = diagp.tile([128, 128], F32R, tag="diag")
                    if s % 4 == 1:
                        # a few diag builds on ScalarE to balance engines
                        nc.scalar.mul(out=diag[:], in_=ident[:],
                                      mul=w[:, s:s + 1])
                    else:
                        nc.vector.tensor_scalar(
                            out=diag[:], in0=ident[:],
                            scalar1=w[:, s:s + 1], scalar2=None,
                            op0=mybir.AluOpType.mult,
                        )
                    for c in range(2):
                        nc.tensor.matmul(
                            out=m_ps[c][:],
                            lhsT=diag[:],
                            rhs=hslice(k, s)[:, c * 512:(c + 1) * 512],
                            start=(i == 0),
                            stop=(i == S - 1),
                        )

                ot = outpool.tile([128, D], F32, tag="ot")
                nc.vector.tensor_copy(out=ot[:, 0:512], in_=m_ps[0][:])
                nc.scalar.copy(out=ot[:, 512:1024], in_=m_ps[1][:])
                nc.gpsimd.dma_start(out=out[t0:t0 + TS, :], in_=ot[:])

            # software-pipelined emission: DMAs one supertile ahead of
            # compute so ring configs never sit behind compute in the
            # per-engine queues
            emit_dma(0)
            for k in range(N_SUPER):
                if k + 1 < N_SUPER:
                    emit_dma(k + 1)
                emit_compute(k)

    nc.compile()
    return nc


_NC = None


def _get_program():
    global _NC
    if _NC is None:
        _NC = _build_program()
    return _NC


def kernel(history, query, rms_weight):
    history = np.asarray(history, dtype=np.float32)
    query = np.asarray(query, dtype=np.float32)
    rms_weight = np.asarray(rms_weight, dtype=np.float32)
    assert history.shape == (S, B, T, D), history.shape

    nc = _get_program()
    ident = np.eye(128, dtype=np.float32)

    in_maps = []
    for c in range(N_CORES):
        b, h = c // 2, c % 2
        shard = np.ascontiguousarray(history[:, b, h * TC:(h + 1) * TC, :])
        in_maps.append({
            "hist": shard,
            "query": query,
            "rms_weight": rms_weight,
            "ident": ident,
        })

    res = bass_utils.run_bass_kernel_spmd(nc, in_maps, list(range(N_CORES)))

    out = np.empty((B, T, D), dtype=np.float32)
    for c in range(N_CORES):
        b, h = c // 2, c % 2
        out[b, h * TC:(h + 1) * TC, :] = res.results[c]["out"]
    return out
